# revision 1
# baseline (speedup 1.0000x reference)
"""8-core Trainium2 Bass kernel for nn_MixModel (GCN mix model) — v3.

Sharding: nodes dealt round-robin by in-degree rank to 8 cores; each core owns
NLOC = ceil((ceil(N/8)+1)/128)*128 local rows (>=1 zero pad row reused as the
ELL gather-pad target).

Algebra used:
 - GCN messages factorize: msg = (h*dis)[src], output scaled by dis[dst]; the
   self-loop term is a local-tile add (pi-order stages) or an extra ELL slot
   (hop stage). Aggregation = unweighted padded-ELL gather+sum of pre-scaled
   table rows.
 - segsum and the layer matmul commute: sum((z@W*dis)[src]) =
   sum((z*dis)[src]) @ W — so cores AllGather the *scaled activations* and the
   per-layer matmul runs on the 98 aggregated dst tiles.
 - good/bad paths share edge sets -> gather concatenated 256-wide tables.
 - the permuted-input path's first-layer table is a cheap local permutation
   gather of the xW1' table (12.5k rows), not a per-edge pass.

Gather engine: per-slot indirect_dma_start (SWDGE, ~1.1us/call for 128 rows;
measured DSP descriptor-gen is ~7-11ns/row for every SWDGE mechanism, so the
per-slot ELL at 1.07x padding beats int16-chunked dma_gather at 2.5x padding).
The self-loop term of the pi-order stages is a local-tile add instead of an
ELL slot (saves ~300 calls). Shared tables use a piece-major layout
([piece][core][rows], NSPL=8) so each AllGather is split into 8 contiguous
pieces issued as their source tiles complete, hiding collective latency
under the gather stream. x is staged tile-major ([nt*128, 512] blocks) so S0
loads are single 256KB DMAs with 2KB rows spread across all DMA engines.

Stages (per core):
  S0   xW1' shard = (x_sh @ W1) * dis_sh
  AG0  AllGather -> XW [NG,128]
  S2   T1 shard = [xW1'_loc | gather(XW, gperm)*ratio] ; AG1 -> T1 [NG,256]
  G1   ELL gather T1 -> zd = relu(dis^2 * sum)  (= z1*dis)      -> AG2 ZD
  G2   ELL gather ZD -> S ; e1{,b} = relu(dis * (S_h @ W2)) ;
       ship [e1*dish|e1b*dish] -> AG3a E1H ; [e1*dis] -> AG3b E1D ; e1 local
  G3   ELL gather E1H (hop order) -> embed2{,b} = dish * (S_h @ W3) -> E2h
  S12  MLP: embed3 = relu(e1@M1)@M2 ; tvec = embed3@Wd0
  S11  realign E2h to pi order ; scores = sigmoid(rowsum(tvec * e2{,b}))
  G4   ELL gather E1D -> cls = (dis*sum)@Wc -> OUT[:, :10]
"""

import numpy as np

import concourse.bacc as bacc
import concourse.bass as bass
import concourse.mybir as mybir
import concourse.tile as tile
from concourse import bass_utils
from concourse.masks import make_identity

P = 128
F32 = mybir.dt.float32
I32 = mybir.dt.int32
I16 = mybir.dt.int16
AF = mybir.ActivationFunctionType
ALU = mybir.AluOpType
TDT = mybir.dt.bfloat16  # transport/table dtype

# ----------------------------------------------------------------- host prep


def _ell_build(src_g, dst_core, dst_loc, self_g, n_cores, nloc, padrow):
    """Shared-K ELL: returns (K per tile, per-core int32 [P, sum(K)] arrays,
    p-major-global: element [p, koff[t]+k] = slot k of local row t*128+p)."""
    nt = nloc // P
    counts = np.zeros((n_cores, nloc), np.int64)
    np.add.at(counts, (dst_core, dst_loc), 1)
    n_self = 0 if self_g is None else 1
    cmax = counts.reshape(n_cores, nt, P).max(axis=(0, 2))
    K = (cmax + n_self).astype(np.int64)
    order = np.lexsort((dst_loc, dst_core))
    sc, sl, sg = dst_core[order], dst_loc[order], src_g[order]
    key = sc.astype(np.int64) * nloc + sl
    is_start = np.r_[True, key[1:] != key[:-1]] if len(key) else np.array([], bool)
    run_starts = np.flatnonzero(is_start)
    run_len = np.diff(np.r_[run_starts, len(key)])
    pos_in_run = np.arange(len(key)) - np.repeat(run_starts, run_len)
    koff = np.r_[0, np.cumsum(K)]
    sk = int(koff[-1])
    idx_arrs = []
    for c in range(n_cores):
        arr = np.full((P, sk), padrow[c], np.int64)
        m = sc == c
        loc, pos, gidx = sl[m], pos_in_run[m], sg[m]
        t = loc // P
        p = loc % P
        arr[p, koff[t] + pos + n_self] = gidx
        if n_self:
            allt = np.arange(nloc) // P
            allp = np.arange(nloc) % P
            arr[allp, koff[allt]] = self_g[c]
        idx_arrs.append(arr.astype(np.int32))
    return K.tolist(), idx_arrs


def _plane(vals_loc, nt):
    """[nloc] local-row vector -> [P, nt] plane (local row t*128+p -> [p, t])."""
    return np.ascontiguousarray(vals_loc.reshape(nt, P).T)


def prep(inputs, n_cores=8):
    x = np.asarray(inputs["x"], np.float32)
    ei = np.asarray(inputs["edge_index"], np.int64)
    eih = np.asarray(inputs["edge_index_hop"], np.int64)
    perm = np.asarray(inputs["perm"], np.int64)
    W1 = np.asarray(inputs["W1"], np.float32)
    W2 = np.asarray(inputs["W2"], np.float32)
    W3 = np.asarray(inputs["W3"], np.float32)
    M1 = np.asarray(inputs["M1"], np.float32)
    M2 = np.asarray(inputs["M2"], np.float32)
    Wc = np.asarray(inputs["Wc"], np.float32)
    Wd0 = np.asarray(inputs["Wd"], np.float32)[0]
    for bname in ("b1", "b2", "b3", "mb1", "mb2", "bc"):
        assert np.abs(np.asarray(inputs[bname])).max() == 0.0, (
            f"nonzero bias {bname} not supported by this kernel build"
        )

    N, n_feat = x.shape
    D = W1.shape[1]
    ncls = Wc.shape[1]
    max_real = -(-N // n_cores)
    nloc = -(-(max_real + 1) // P) * P
    nt = nloc // P
    ng = n_cores * nloc

    deg = np.bincount(ei[1], minlength=N).astype(np.float32) + 1.0
    degh = np.bincount(eih[1], minlength=N).astype(np.float32) + 1.0
    dis = 1.0 / np.sqrt(deg)
    dish = 1.0 / np.sqrt(degh)

    order = np.argsort(-deg, kind="stable")
    core_of = np.empty(N, np.int64)
    loc_of = np.empty(N, np.int64)
    core_of[order] = np.arange(N) % n_cores
    loc_of[order] = np.arange(N) // n_cores

    # piece-major global table layout: [piece][core][piece-local rows], so
    # each split-AllGather piece lands contiguously in the shared tables.
    NSPL = 8
    pr0 = np.array([(nt * i // NSPL) * P for i in range(NSPL + 1)], np.int64)

    def glmap(c, r):
        p = np.searchsorted(pr0, r, side="right") - 1
        return 8 * pr0[p] + c * (pr0[p + 1] - pr0[p]) + (r - pr0[p])

    gl = glmap(core_of, loc_of)
    padrow = [int(glmap(np.int64(c), np.int64(nloc - 1))) for c in range(n_cores)]

    nat = np.full((n_cores, nloc), -1, np.int64)
    nat[core_of, loc_of] = np.arange(N)

    # hop order: per-core resort by hop degree desc (pads last)
    hkey = np.where(nat >= 0, -degh[np.maximum(nat, 0)], 1.0)
    hord = np.argsort(hkey, axis=1, kind="stable")
    hpos = np.argsort(hord, axis=1)

    allg = glmap(
        np.repeat(np.arange(n_cores), nloc), np.tile(np.arange(nloc), n_cores)
    ).reshape(n_cores, nloc)
    selfg_pi = np.where(nat >= 0, allg, np.array(padrow)[:, None])
    K1, idx1 = _ell_build(
        gl[ei[0]], core_of[ei[1]], loc_of[ei[1]], None, n_cores, nloc, padrow
    )
    selfg_h = np.take_along_axis(selfg_pi, hord, axis=1)
    K3, idx3 = _ell_build(
        gl[eih[0]],
        core_of[eih[1]],
        hpos[core_of[eih[1]], loc_of[eih[1]]],
        selfg_h,
        n_cores,
        nloc,
        padrow,
    )

    in_maps = []
    for c in range(n_cores):
        natc = nat[c]
        real = natc >= 0
        xs = np.zeros((nloc, n_feat), np.float32)
        xs[real] = x[natc[real]]
        dis_c = np.ones(nloc, np.float32)
        dis_c[real] = dis[natc[real]]
        dish_pi = np.ones(nloc, np.float32)
        dish_pi[real] = dish[natc[real]]
        dishh = np.ones(nloc, np.float32)
        hnat = natc[hord[c]]
        hreal = hnat >= 0
        dishh[hreal] = dish[hnat[hreal]]
        gperm = np.full(nloc, padrow[c], np.int64)
        ratio = np.ones(nloc, np.float32)
        pv = perm[natc[real]]
        gperm[real] = gl[pv]
        ratio[real] = dis[natc[real]] / dis[pv]
        in_maps.append(
            {
                "xTb": np.ascontiguousarray(
                    xs.reshape(nt, P, 4, P).transpose(0, 3, 2, 1).reshape(nt * P, 4 * P)
                ),
                "dis_p": _plane(dis_c, nt),
                "dis2_p": _plane(dis_c * dis_c, nt),
                "dishp_p": _plane(dish_pi, nt),
                "dishh_p": _plane(dishh, nt),
                "ratio_p": _plane(ratio, nt),
                "gperm_p": _plane(gperm.astype(np.int32), nt),
                "idxR_p": _plane(hpos[c].astype(np.int32), nt),
                "idx1": idx1[c],
                "idx3": idx3[c],
                "W1": W1,
                "W2": W2,
                "W3": W3,
                "M1": M1,
                "M2": M2,
                "Wd0": Wd0,
                "Wc": np.ascontiguousarray(Wc),
            }
        )

    meta = dict(
        n_cores=n_cores,
        nloc=nloc,
        nt=nt,
        ng=ng,
        n_feat=n_feat,
        D=D,
        ncls=ncls,
        K1=K1,
        K3=K3,
        core_of=core_of,
        loc_of=loc_of,
    )
    return in_maps, meta


# ------------------------------------------------------------- device build


def build(meta):
    n_cores = meta["n_cores"]
    nloc, nt, ng = meta["nloc"], meta["nt"], meta["ng"]
    n_feat, D, ncls = meta["n_feat"], meta["D"], meta["ncls"]
    K1, K3 = meta["K1"], meta["K3"]
    DD = 2 * D
    nfc = n_feat // P
    sk1, sk3 = sum(K1), sum(K3)
    groups = [list(range(n_cores))]

    nc = bacc.Bacc("TRN2", debug=False, num_devices=n_cores)
    shared = "Shared" if n_cores > 4 else "Local"

    xTb = nc.dram_tensor("xTb", [nt * P, 4 * P], F32, kind="ExternalInput")
    dis_p = nc.dram_tensor("dis_p", [P, nt], F32, kind="ExternalInput")
    dis2_p = nc.dram_tensor("dis2_p", [P, nt], F32, kind="ExternalInput")
    dishp_p = nc.dram_tensor("dishp_p", [P, nt], F32, kind="ExternalInput")
    dishh_p = nc.dram_tensor("dishh_p", [P, nt], F32, kind="ExternalInput")
    ratio_p = nc.dram_tensor("ratio_p", [P, nt], F32, kind="ExternalInput")
    gperm_p = nc.dram_tensor("gperm_p", [P, nt], I32, kind="ExternalInput")
    idxR_p = nc.dram_tensor("idxR_p", [P, nt], I32, kind="ExternalInput")
    idx1 = nc.dram_tensor("idx1", [P, sk1], I32, kind="ExternalInput")
    idx3 = nc.dram_tensor("idx3", [P, sk3], I32, kind="ExternalInput")
    W1 = nc.dram_tensor("W1", [n_feat, D], F32, kind="ExternalInput")
    W2 = nc.dram_tensor("W2", [D, D], F32, kind="ExternalInput")
    W3 = nc.dram_tensor("W3", [D, D], F32, kind="ExternalInput")
    M1 = nc.dram_tensor("M1", [D, D], F32, kind="ExternalInput")
    M2 = nc.dram_tensor("M2", [D, D], F32, kind="ExternalInput")
    Wd0 = nc.dram_tensor("Wd0", [D, D], F32, kind="ExternalInput")
    Wc = nc.dram_tensor("Wc", [D, ncls], F32, kind="ExternalInput")
    out = nc.dram_tensor("out", [nloc, ncls + 2], F32, kind="ExternalOutput")

    xw_s = nc.dram_tensor("xw_s", [nloc, D], TDT, kind="Internal")
    XW = nc.dram_tensor("XW", [ng, D], TDT, kind="Internal", addr_space=shared)
    t1_s = nc.dram_tensor("t1_s", [nloc, DD], TDT, kind="Internal")
    T1 = nc.dram_tensor("T1", [ng, DD], TDT, kind="Internal", addr_space=shared)
    zd_s = nc.dram_tensor("zd_s", [nloc, DD], TDT, kind="Internal")
    ZD = nc.dram_tensor("ZD", [ng, DD], TDT, kind="Internal", addr_space=shared)
    e1_s = nc.dram_tensor("e1_s", [nloc, D], F32, kind="Internal")
    e1h_s = nc.dram_tensor("e1h_s", [nloc, DD], TDT, kind="Internal")
    e1d_s = nc.dram_tensor("e1d_s", [nloc, D], TDT, kind="Internal")
    E1H = nc.dram_tensor("E1H", [ng, DD], TDT, kind="Internal", addr_space=shared)
    E1D = nc.dram_tensor("E1D", [ng, D], TDT, kind="Internal", addr_space=shared)
    E2h = nc.dram_tensor("E2h", [nloc, DD], F32, kind="Internal")
    TV = nc.dram_tensor("TV", [nloc, D], F32, kind="Internal")

    with tile.TileContext(nc) as tc:
        with (
            tc.tile_pool(name="const", bufs=1) as constp,
            tc.tile_pool(name="gath", bufs=6) as gathp,
            tc.tile_pool(name="work", bufs=3) as workp,
            tc.tile_pool(name="outp", bufs=3) as outp,
            tc.tile_pool(name="psum", bufs=2, space="PSUM") as psp,
        ):
            ident = constp.tile([P, P], F32)
            make_identity(nc, ident[:])

            # resident planes + indices
            def res(t_dram, w, dt=F32, name=None):
                tl = constp.tile([P, w], dt, name=name)
                nc.sync.dma_start(tl[:], t_dram.ap())
                return tl

            disq = res(dis_p, nt, name="disq")
            dis2q = res(dis2_p, nt, name="dis2q")
            dishpq = res(dishp_p, nt, name="dishpq")
            dishhq = res(dishh_p, nt, name="dishhq")
            ratioq = res(ratio_p, nt, name="ratioq")
            gpermq = res(gperm_p, nt, I32, name="gpermq")
            idxRq = res(idxR_p, nt, I32, name="idxRq")
            idx1q = res(idx1, sk1, I32, name="idx1q")
            idx3q = res(idx3, sk3, I32, name="idx3q")

            w1t = [
                constp.tile([P, D], F32, name=f"w1t_{i}") for i in range(nfc)
            ]
            for i in range(nfc):
                nc.sync.dma_start(w1t[i][:], W1.ap()[i * P : (i + 1) * P])
            w2t = res(W2, D, name="w2t")
            w3t = res(W3, D, name="w3t")
            m1t = res(M1, D, name="m1t")
            m2t = res(M2, D, name="m2t")
            wdt = res(Wd0, D, name="wdt")
            wct = res(Wc, ncls, name="wct")

            def rows(t):
                return slice(t * P, (t + 1) * P)

            def col(plane, t):
                return plane[:, t : t + 1]

            NSPL = 8
            bound = [nt * (i + 1) // NSPL - 1 for i in range(NSPL)]

            def ag_piece(src, dst, piece):
                r0 = (nt * piece // NSPL) * P
                r1 = (nt * (piece + 1) // NSPL) * P
                nc.gpsimd.collective_compute(
                    "AllGather",
                    ALU.bypass,
                    replica_groups=groups,
                    ins=[src[r0:r1].opt()],
                    outs=[dst[n_cores * r0 : n_cores * r1].opt()],
                )

            # ---- S0: xW1' shard
            sp = 0
            for t in range(nt):
                ps = psp.tile([P, D], F32, tag="mm")
                xt = workp.tile([P, nfc * P], F32, tag="xt")
                nc.scalar.dma_start(xt[:], xTb.ap()[rows(t)])
                for i in range(nfc):
                    nc.tensor.matmul(
                        out=ps[:],
                        lhsT=xt[:, i * P : (i + 1) * P],
                        rhs=w1t[i][:],
                        start=(i == 0),
                        stop=(i == nfc - 1),
                    )
                o = outp.tile([P, D], TDT, tag="s0")
                nc.vector.tensor_scalar_mul(o[:], ps[:], col(disq, t))
                nc.sync.dma_start(xw_s.ap()[rows(t)], o[:])
                nc.sync.dma_start(t1_s.ap()[rows(t), 0:D], o[:])
                if t == bound[sp]:
                    ag_piece(xw_s, XW, sp)
                    sp += 1

            # ---- S2: T1 shard (bad half; good half written by S0)
            sp = 0
            for t in range(nt):
                g = gathp.tile([P, D], TDT, tag="g2")
                nc.gpsimd.indirect_dma_start(
                    out=g[:],
                    out_offset=None,
                    in_=XW.ap(),
                    in_offset=bass.IndirectOffsetOnAxis(ap=col(gpermq, t), axis=0),
                )
                o = outp.tile([P, D], TDT, tag="s2")
                nc.vector.tensor_scalar_mul(o[:], g[:], col(ratioq, t))
                nc.sync.dma_start(t1_s.ap()[rows(t), D:DD], o[:])
                if t == bound[sp]:
                    ag_piece(t1_s, T1, sp)
                    sp += 1

            # ---- per-slot indirect ELL gather driver -------------------
            # one indirect DMA per (tile, slot): [P,1] offset column gathers
            # 128 rows; slots accumulate via a vector reduce. The self-loop
            # term is a local-tile add (local_s) instead of an ELL slot.
            def ell_run(table, width, Ks, idxq, local_s, tail):
                koff = 0
                for t in range(nt):
                    K = Ks[t]
                    g = gathp.tile([P, K * width], TDT, tag="ge")
                    for k in range(K):
                        nc.gpsimd.indirect_dma_start(
                            out=g[:, k * width : (k + 1) * width],
                            out_offset=None,
                            in_=table.ap(),
                            in_offset=bass.IndirectOffsetOnAxis(
                                ap=idxq[:, koff + k : koff + k + 1], axis=0
                            ),
                        )
                    koff += K
                    s = workp.tile([P, width], F32, tag="se")
                    if K == 1:
                        nc.vector.tensor_copy(s[:], g[:])
                    else:
                        nc.vector.tensor_reduce(
                            out=s[:],
                            in_=g[:].rearrange("p (k d) -> p d k", k=K),
                            axis=mybir.AxisListType.X,
                            op=ALU.add,
                        )
                    if local_s is not None:
                        li = workp.tile([P, width], TDT, tag="sl")
                        nc.scalar.dma_start(li[:], local_s.ap()[rows(t)])
                        nc.vector.tensor_tensor(
                            out=s[:], in0=s[:], in1=li[:], op=ALU.add
                        )
                    tail(t, s)

            # ---- G1: zd = relu(dis2 * sum) -> zd_s
            spl = [0]

            def g1_tail(t, s):
                o = outp.tile([P, DD], TDT, tag="ze")
                nc.vector.tensor_scalar(
                    o[:], s[:], col(dis2q, t), 0.0, ALU.mult, ALU.max
                )
                nc.sync.dma_start(zd_s.ap()[rows(t)], o[:])
                if t == bound[spl[0]]:
                    ag_piece(zd_s, ZD, spl[0])
                    spl[0] += 1

            ell_run(T1, DD, K1, idx1q, t1_s, g1_tail)

            # ---- G2: S @ W2, three shipped variants
            def g2_tail(t, s):
                e1h = outp.tile([P, DD], TDT, tag="e1h")
                e1d = outp.tile([P, D], TDT, tag="e1d")
                e1p = outp.tile([P, D], F32, tag="e1p")
                for h in range(2):
                    tp = psp.tile([P, P], F32, tag="t", bufs=3)
                    nc.tensor.transpose(
                        out=tp[:], in_=s[:, h * D : (h + 1) * D], identity=ident[:]
                    )
                    tps = workp.tile([P, P], F32, tag="tps")
                    nc.vector.tensor_copy(tps[:], tp[:])
                    mm = psp.tile([P, D], F32, tag="m", bufs=3)
                    nc.tensor.matmul(
                        out=mm[:], lhsT=tps[:], rhs=w2t[:], start=True, stop=True
                    )
                    # e1 = relu(dis * mm)
                    eh = workp.tile([P, D], F32, tag="eh")
                    nc.vector.tensor_scalar(
                        eh[:], mm[:], col(disq, t), 0.0, ALU.mult, ALU.max
                    )
                    nc.vector.tensor_scalar_mul(
                        e1h[:, h * D : (h + 1) * D], eh[:], col(dishpq, t)
                    )
                    if h == 0:
                        nc.vector.tensor_copy(e1p[:], eh[:])
                        nc.vector.tensor_scalar_mul(e1d[:], eh[:], col(disq, t))
                nc.sync.dma_start(e1_s.ap()[rows(t)], e1p[:])
                nc.sync.dma_start(e1h_s.ap()[rows(t)], e1h[:])
                nc.sync.dma_start(e1d_s.ap()[rows(t)], e1d[:])
                if t == bound[spl[0]]:
                    ag_piece(e1h_s, E1H, spl[0])
                    ag_piece(e1d_s, E1D, spl[0])
                    spl[0] += 1

            spl[0] = 0
            ell_run(ZD, DD, K1, idx1q, zd_s, g2_tail)

            # ---- S12: MLP + tvec (local, overlaps with AG3/G3)
            for t in range(nt):
                et = workp.tile([P, D], F32, tag="ml_in")
                nc.sync.dma_start(et[:], e1_s.ap()[rows(t)])
                tp = psp.tile([P, P], F32, tag="t", bufs=3)
                nc.tensor.transpose(out=tp[:], in_=et[:], identity=ident[:])
                tps = workp.tile([P, P], F32, tag="tps")
                nc.vector.tensor_copy(tps[:], tp[:])
                mm = psp.tile([P, D], F32, tag="m", bufs=3)
                nc.tensor.matmul(out=mm[:], lhsT=tps[:], rhs=m1t[:], start=True, stop=True)
                u = workp.tile([P, D], F32, tag="ml_u")
                nc.scalar.activation(u[:], mm[:], AF.Relu)
                tp2 = psp.tile([P, P], F32, tag="t", bufs=3)
                nc.tensor.transpose(out=tp2[:], in_=u[:], identity=ident[:])
                tps2 = workp.tile([P, P], F32, tag="tps")
                nc.vector.tensor_copy(tps2[:], tp2[:])
                mm2 = psp.tile([P, D], F32, tag="m", bufs=3)
                nc.tensor.matmul(
                    out=mm2[:], lhsT=tps2[:], rhs=m2t[:], start=True, stop=True
                )
                e3 = workp.tile([P, D], F32, tag="ml_e3")
                nc.vector.tensor_copy(e3[:], mm2[:])
                tp3 = psp.tile([P, P], F32, tag="t", bufs=3)
                nc.tensor.transpose(out=tp3[:], in_=e3[:], identity=ident[:])
                tps3 = workp.tile([P, P], F32, tag="tps")
                nc.vector.tensor_copy(tps3[:], tp3[:])
                mm3 = psp.tile([P, D], F32, tag="m", bufs=3)
                nc.tensor.matmul(
                    out=mm3[:], lhsT=tps3[:], rhs=wdt[:], start=True, stop=True
                )
                tv = outp.tile([P, D], F32, tag="ml_tv")
                nc.vector.tensor_copy(tv[:], mm3[:])
                nc.sync.dma_start(TV.ap()[rows(t)], tv[:])

            # ---- G3: embed2{,b} = dishh * (S_h @ W3) -> E2h (hop order)
            def g3_tail(t, s):
                e2 = outp.tile([P, DD], F32, tag="e2")
                for h in range(2):
                    tp = psp.tile([P, P], F32, tag="t", bufs=3)
                    nc.tensor.transpose(
                        out=tp[:], in_=s[:, h * D : (h + 1) * D], identity=ident[:]
                    )
                    tps = workp.tile([P, P], F32, tag="tps")
                    nc.vector.tensor_copy(tps[:], tp[:])
                    mm = psp.tile([P, D], F32, tag="m", bufs=3)
                    nc.tensor.matmul(
                        out=mm[:], lhsT=tps[:], rhs=w3t[:], start=True, stop=True
                    )
                    nc.vector.tensor_scalar_mul(
                        e2[:, h * D : (h + 1) * D], mm[:], col(dishhq, t)
                    )
                nc.sync.dma_start(E2h.ap()[rows(t)], e2[:])

            ell_run(E1H, DD, K3, idx3q, None, g3_tail)

            # ---- S11 + S13: realign + scores
            for t in range(nt):
                e2 = gathp.tile([P, DD], F32, tag="gr")
                nc.gpsimd.indirect_dma_start(
                    out=e2[:],
                    out_offset=None,
                    in_=E2h.ap(),
                    in_offset=bass.IndirectOffsetOnAxis(ap=col(idxRq, t), axis=0),
                )
                tv = workp.tile([P, D], F32, tag="sc_tv")
                nc.sync.dma_start(tv[:], TV.ap()[rows(t)])
                pr = workp.tile([P, DD], F32, tag="sc_pr")
                nc.vector.tensor_mul(pr[:, 0:D], tv[:], e2[:, 0:D])
                nc.vector.tensor_mul(pr[:, D:DD], tv[:], e2[:, D:DD])
                rs = workp.tile([P, 2], F32, tag="sc_rs")
                nc.vector.tensor_reduce(
                    out=rs[:],
                    in_=pr[:].rearrange("p (h d) -> p h d", h=2),
                    axis=mybir.AxisListType.X,
                    op=ALU.add,
                )
                sg = outp.tile([P, 2], F32, tag="sc_sg")
                nc.scalar.activation(sg[:], rs[:], AF.Sigmoid)
                nc.sync.dma_start(out.ap()[rows(t), ncls : ncls + 2], sg[:])

            # ---- G4: cls = (dis * sum) @ Wc -> out[:, :ncls]
            def g4_tail(t, s):
                sc_ = workp.tile([P, D], F32, tag="c_s")
                nc.vector.tensor_scalar_mul(sc_[:], s[:], col(disq, t))
                tp = psp.tile([P, P], F32, tag="t", bufs=3)
                nc.tensor.transpose(out=tp[:], in_=sc_[:], identity=ident[:])
                tps = workp.tile([P, P], F32, tag="tps")
                nc.vector.tensor_copy(tps[:], tp[:])
                mm = psp.tile([P, ncls], F32, tag="m", bufs=3)
                nc.tensor.matmul(out=mm[:], lhsT=tps[:], rhs=wct[:], start=True, stop=True)
                o = outp.tile([P, ncls], F32, tag="c_o")
                nc.vector.tensor_copy(o[:], mm[:])
                nc.sync.dma_start(out.ap()[rows(t), 0:ncls], o[:])

            ell_run(E1D, D, K1, idx1q, e1d_s, g4_tail)

    nc.compile()
    return nc


def assemble(results, meta):
    n_cores = meta["n_cores"]
    N = len(meta["core_of"])
    ncls = meta["ncls"]
    out = np.empty((N, ncls + 2), np.float32)
    for c in range(n_cores):
        oc = results[c]["out"]
        m = meta["core_of"] == c
        out[m] = oc[meta["loc_of"][m]]
    return out


# ------------------------------------------------------------------ entry


_CACHE = {}
TRACE = False
LAST_RES = None


def kernel(**inputs):
    """Full-input entry point: shards across 8 NeuronCores internally.

    Expects the nn_MixModel input dict (x, edge_index, edge_index_hop, y,
    perm, W1..Wd); returns the full [N, n_cls+2] float32 output.
    """
    n_cores = 8
    in_maps, meta = prep(inputs, n_cores)
    key = (meta["nloc"], tuple(meta["K1"]), tuple(meta["K3"]))
    nc = _CACHE.get(key)
    if nc is None:
        nc = build(meta)
        _CACHE[key] = nc
    res = bass_utils.run_bass_kernel_spmd(
        nc, in_maps, core_ids=list(range(n_cores)), trace=TRACE
    )
    global LAST_RES
    LAST_RES = res
    return assemble(res.results, meta)



# revision 12
# speedup vs baseline: 1.4471x; 1.4471x over previous
"""8-core Trainium2 Bass kernel for nn_MixModel (GCN mix model) — v4.

v4 replaces v3's per-slot indirect_dma_start ELL gathers (994ns fixed cost per
128-row call = 8.5ns/row, single SWDGE queue ~65GB/s) with bulk dma_gather on
4 SWDGE queues (~175GB/s sustained on random 512B rows). dma_gather requires
int16 indices -> tables are split into 4 chunks (<=32768 rows, aligned to
AllGather piece pairs); per-(tile,chunk) ELL rectangles are padded to the
chunk max. Padding is cut from 2.2x to ~1.5x by grouping dst rows into tiles
by per-chunk in-degree profile (balanced KD split within each (core, quarter)
so table chunk membership stays fixed).

Other v4 changes vs v3:
 - bad path's first layer is computed from host-permuted x (xTb carries
   good|bad halves): kills AG0 (XW AllGather) + the gperm/ratio gather.
 - G3's hop self-loop slot becomes a local dma_gather (hord reorder of
   e1h_s) + local-tile add, so ELL rectangles stay chunk-pure.
 - S11's realign and the hop reorder use local (unchunked) dma_gather.
 - int16 wrapped index planes are streamed per tile group (not resident).

Stages (per core):
  S0   T1 shard = [(x@W1)*dis | (x[perm]@W1)*dis] ; AG1 -> T1 [NG,256]
  G1   ELL gather T1 -> zd = relu(dis^2 * (sum + t1_s))      -> AG2 ZD
  G2   ELL gather ZD -> S ; e1{,b} = relu(dis * (S_h @ W2)) ;
       ship [e1*dish|e1b*dish] -> AG3a E1H ; [e1*dis] -> AG3b E1D ; e1 local
  H0   e1h_h = e1h_s[hord] (local gather)
  G3   ELL gather E1H (hop order) + e1h_h -> embed2{,b} -> E2h (bf16)
  S12  MLP: embed3 = relu(e1@M1)@M2 ; tvec = embed3@Wd0
  S11  realign E2h to pi order (local gather); scores = sigmoid(...)
  G4   ELL gather E1D + e1d_s -> cls = (dis*sum)@Wc -> OUT[:, :10]
"""

import numpy as np

import concourse.bacc as bacc
import concourse.bass as bass
import concourse.mybir as mybir
import concourse.tile as tile
from concourse import bass_utils
from concourse.masks import make_identity

P = 128
F32 = mybir.dt.float32
I32 = mybir.dt.int32
I16 = mybir.dt.int16
AF = mybir.ActivationFunctionType
ALU = mybir.AluOpType
TDT = mybir.dt.bfloat16  # transport/table dtype

NCHUNK = 4
NQ = 4          # SWDGE queues
IDX_GRP = 16    # tiles per streamed idx-plane group

# ----------------------------------------------------------------- host prep


def _wrap16(flat):
    """[n] int64 -> wrapped [128, n//16] int16 (replicated across 8 groups)."""
    n = len(flat)
    w = flat.reshape(n // 16, 16).T.astype(np.int16)
    return np.tile(w, (8, 1))


def _kd_group(pr, n_tiles):
    """Split len(pr) rows into n_tiles groups of P by cycling-dim median.

    The split-dim sequence is FIXED (depth % NCHUNK) so all cores produce
    aligned profile regions per tile index -- the shared-K (max over cores)
    padding stays close to the per-core padding.
    """
    out = []

    def split(ii, k, d):
        if k == 1:
            out.append(ii)
            return
        k1 = k // 2
        o = np.argsort(pr[ii, d % NCHUNK], kind="stable")
        split(ii[o[: k1 * P]], k1, d + 1)
        split(ii[o[k1 * P :]], k - k1, d + 1)

    split(np.arange(len(pr)), n_tiles, 0)
    return out


def _ell_build_chunked(src_g, dst_core, dst_loc, n_cores, nloc, cbounds, padg):
    """Chunked shared-K ELL.

    Returns (K [nt, NCHUNK] per-chunk slot counts (same for all cores),
    per-core wrapped int16 idx planes [128, sum(K)*8] with chunk-local
    indices, slot-major per rectangle). padg[c][ch] = global row of a zero
    row inside chunk ch for core c.
    """
    nt = nloc // P
    cid = np.searchsorted(cbounds, src_g, side="right") - 1
    counts = np.zeros((n_cores, nloc, NCHUNK), np.int64)
    np.add.at(counts, (dst_core, dst_loc, cid), 1)
    K = counts.reshape(n_cores, nt, P, NCHUNK).max(axis=(0, 2))  # [nt, NCHUNK]

    order = np.lexsort((src_g, dst_loc, dst_core))
    sc, sl, sg = dst_core[order], dst_loc[order], src_g[order]
    scid = cid[order]
    key = ((sc * nloc + sl) * NCHUNK + scid).astype(np.int64)
    is_start = np.r_[True, key[1:] != key[:-1]] if len(key) else np.array([], bool)
    run_starts = np.flatnonzero(is_start)
    run_len = np.diff(np.r_[run_starts, len(key)])
    pos_in_run = np.arange(len(key)) - np.repeat(run_starts, run_len)

    koff = np.zeros((nt, NCHUNK), np.int64)
    flat = np.cumsum(K.reshape(-1))
    koff.reshape(-1)[1:] = flat[:-1]
    sk = int(flat[-1])

    planes = []
    for c in range(n_cores):
        # flat slot-major list per rectangle: entry (t, ch, k, p)
        fl = np.zeros(sk * P, np.int64)
        # fill pads first
        for t in range(nt):
            for ch in range(NCHUNK):
                base = koff[t, ch] * P
                fl[base : base + K[t, ch] * P] = padg[c][ch] - cbounds[ch]
        m = sc == c
        loc, pos, gidx, ch = sl[m], pos_in_run[m], sg[m], scid[m]
        t = loc // P
        p = loc % P
        fl[(koff[t, ch] + pos) * P + p] = gidx - cbounds[ch]
        planes.append(_wrap16(fl))
    return K, planes


def _plane(vals_loc, nt):
    return np.ascontiguousarray(vals_loc.reshape(nt, P).T)


def prep(inputs, n_cores=8):
    x = np.asarray(inputs["x"], np.float32)
    ei = np.asarray(inputs["edge_index"], np.int64)
    eih = np.asarray(inputs["edge_index_hop"], np.int64)
    perm = np.asarray(inputs["perm"], np.int64)
    W1 = np.asarray(inputs["W1"], np.float32)
    W2 = np.asarray(inputs["W2"], np.float32)
    W3 = np.asarray(inputs["W3"], np.float32)
    M1 = np.asarray(inputs["M1"], np.float32)
    M2 = np.asarray(inputs["M2"], np.float32)
    Wc = np.asarray(inputs["Wc"], np.float32)
    Wd0 = np.asarray(inputs["Wd"], np.float32)[0]
    for bname in ("b1", "b2", "b3", "mb1", "mb2", "bc"):
        assert np.abs(np.asarray(inputs[bname])).max() == 0.0, (
            f"nonzero bias {bname} not supported by this kernel build"
        )

    N, n_feat = x.shape
    D = W1.shape[1]
    ncls = Wc.shape[1]
    max_real = -(-N // n_cores)
    nloc = -(-(max_real + 1) // P) * P
    nt = nloc // P
    ng = n_cores * nloc

    deg = np.bincount(ei[1], minlength=N).astype(np.float32) + 1.0
    degh = np.bincount(eih[1], minlength=N).astype(np.float32) + 1.0
    dis = 1.0 / np.sqrt(deg)
    dish = 1.0 / np.sqrt(degh)

    order = np.argsort(-deg, kind="stable")
    core_of = np.empty(N, np.int64)
    loc_of = np.empty(N, np.int64)
    core_of[order] = np.arange(N) % n_cores
    loc_of[order] = np.arange(N) // n_cores

    # piece-major global layout; chunks = piece pairs (<=32768 rows, int16).
    NSPL = 8
    pr0 = np.array([(nt * i // NSPL) * P for i in range(NSPL + 1)], np.int64)
    qb = [0, int(pr0[2]), int(pr0[4]), int(pr0[6]), nloc]

    # remap dense ranks so each quarter keeps >=1 unassigned slot (zero row)
    reserved = [qb[1] - 1, qb[2] - 1, qb[3] - 1]
    avail = np.setdiff1d(np.arange(nloc), np.array(reserved, np.int64))
    loc_of = avail[loc_of]

    def glmap(c, r):
        p = np.searchsorted(pr0, r, side="right") - 1
        return 8 * pr0[p] + c * (pr0[p + 1] - pr0[p]) + (r - pr0[p])

    cbounds = (8 * pr0)[::2][:NCHUNK]  # chunk start rows (global)

    # chunk profile of each dst node (chunk membership is loc-permutation
    # invariant within quarters, so profiles stay exact through regrouping)
    gl0 = glmap(core_of, loc_of)
    cid1 = np.searchsorted(cbounds, gl0[ei[0]], side="right") - 1
    prof1 = np.zeros((N, NCHUNK), np.int32)
    np.add.at(prof1, (ei[1], cid1), 1)

    # regroup: within each (core, quarter), KD-group the real rows by chunk
    # profile; at least one slot per quarter stays unassigned (zero pad row).
    loc_new = np.full(N, -1, np.int64)
    padloc = np.zeros((n_cores, NCHUNK), np.int64)
    for c in range(n_cores):
        nodes_c = np.flatnonzero(core_of == c)
        locs_c = loc_of[nodes_c]
        for qi in range(NCHUNK):
            lo, hi = qb[qi], qb[qi + 1]
            seg = nodes_c[(locs_c >= lo) & (locs_c < hi)]
            navail = hi - lo
            assert len(seg) <= navail - 1, (len(seg), navail)
            ntile = navail // P
            pr = prof1[seg].astype(np.int32)
            prf = np.vstack([pr, np.zeros((navail - len(seg), NCHUNK), np.int32)])
            groups = _kd_group(prf, ntile)
            used = np.zeros(navail, bool)
            pos = lo
            for g in groups:
                real = g[g < len(seg)]
                loc_new[seg[real]] = pos + np.arange(len(real))
                used[pos - lo : pos - lo + len(real)] = True
                pos += len(g)
            free_slots = np.flatnonzero(~used)
            assert len(free_slots) >= 1
            padloc[c, qi] = lo + free_slots[0]
    loc_of = loc_new
    assert (loc_of >= 0).all()

    gl = glmap(core_of, loc_of)
    padg = [[glmap(np.int64(c), np.int64(padloc[c, q])) for q in range(NCHUNK)]
            for c in range(n_cores)]

    nat = np.full((n_cores, nloc), -1, np.int64)
    nat[core_of, loc_of] = np.arange(N)

    # hop order: free per-core regroup by hop chunk profile
    cidh = np.searchsorted(cbounds, gl[eih[0]], side="right") - 1
    profh = np.zeros((N, NCHUNK), np.int32)
    np.add.at(profh, (eih[1], cidh), 1)
    hord = np.empty((n_cores, nloc), np.int64)  # hop row r <- pi-loc hord[c,r]
    for c in range(n_cores):
        natc = nat[c]
        pr = np.zeros((nloc, NCHUNK), np.int32)
        rm = natc >= 0
        pr[rm] = profh[natc[rm]]
        groups = _kd_group(pr, nt)
        perm_rows = np.concatenate(groups)
        hord[c] = perm_rows
    hpos = np.argsort(hord, axis=1)

    K1, idx1p = _ell_build_chunked(
        gl[ei[0]], core_of[ei[1]], loc_of[ei[1]], n_cores, nloc, cbounds, padg
    )
    K3, idx3p = _ell_build_chunked(
        gl[eih[0]],
        core_of[eih[1]],
        hpos[core_of[eih[1]], loc_of[eih[1]]],
        n_cores,
        nloc,
        cbounds,
        padg,
    )

    in_maps = []
    for c in range(n_cores):
        natc = nat[c]
        real = natc >= 0
        xs = np.zeros((nloc, n_feat), np.float32)
        xs[real] = x[natc[real]]
        xb = np.zeros((nloc, n_feat), np.float32)
        xb[real] = x[perm[natc[real]]]
        dis_c = np.ones(nloc, np.float32)
        dis_c[real] = dis[natc[real]]
        dish_pi = np.ones(nloc, np.float32)
        dish_pi[real] = dish[natc[real]]
        dishh = np.ones(nloc, np.float32)
        hnat = natc[hord[c]]
        hreal = hnat >= 0
        dishh[hreal] = dish[hnat[hreal]]
        xcat = np.concatenate([xs, xb], axis=1)  # [nloc, 2*n_feat]
        nfc2 = 2 * n_feat // P
        in_maps.append(
            {
                "xTb": np.ascontiguousarray(
                    xcat.reshape(nt, P, nfc2, P).transpose(0, 3, 2, 1)
                    .reshape(nt * P, nfc2 * P)
                ),
                "dis_p": _plane(dis_c, nt),
                "dis2_p": _plane(dis_c * dis_c, nt),
                "dishp_p": _plane(dish_pi, nt),
                "dishh_p": _plane(dishh, nt),
                "hordw": _wrap16(hord[c]),
                "idxRw": _wrap16(hpos[c]),
                "idx1w": idx1p[c],
                "idx3w": idx3p[c],
                "W1": W1,
                "W2": W2,
                "W3": W3,
                "M1": M1,
                "M2": M2,
                "Wd0": Wd0,
                "Wc": np.ascontiguousarray(Wc),
            }
        )

    meta = dict(
        n_cores=n_cores,
        nloc=nloc,
        nt=nt,
        ng=ng,
        n_feat=n_feat,
        D=D,
        ncls=ncls,
        K1=K1,
        K3=K3,
        pr0=pr0,
        core_of=core_of,
        loc_of=loc_of,
    )
    return in_maps, meta


# ------------------------------------------------------------- device build


def build(meta):
    n_cores = meta["n_cores"]
    nloc, nt, ng = meta["nloc"], meta["nt"], meta["ng"]
    n_feat, D, ncls = meta["n_feat"], meta["D"], meta["ncls"]
    K1, K3 = meta["K1"], meta["K3"]
    pr0 = meta["pr0"]
    DD = 2 * D
    nfc2 = 2 * n_feat // P
    sk1 = int(K1.sum())
    sk3 = int(K3.sum())
    groups = [list(range(n_cores))]
    csz = [int(8 * (pr0[2 * i + 2] - pr0[2 * i])) for i in range(NCHUNK)]
    cbase = [int((8 * pr0)[::2][i]) for i in range(NCHUNK)]

    nc = bacc.Bacc(
        "TRN2", debug=False, num_devices=n_cores, num_swdge_queues=NQ
    )
    shared = "Shared" if n_cores > 4 else "Local"

    xTb = nc.dram_tensor("xTb", [nt * P, nfc2 * P], F32, kind="ExternalInput")
    dis_p = nc.dram_tensor("dis_p", [P, nt], F32, kind="ExternalInput")
    dis2_p = nc.dram_tensor("dis2_p", [P, nt], F32, kind="ExternalInput")
    dishp_p = nc.dram_tensor("dishp_p", [P, nt], F32, kind="ExternalInput")
    dishh_p = nc.dram_tensor("dishh_p", [P, nt], F32, kind="ExternalInput")
    hordw = nc.dram_tensor("hordw", [P, nloc // 16], I16, kind="ExternalInput")
    idxRw = nc.dram_tensor("idxRw", [P, nloc // 16], I16, kind="ExternalInput")
    idx1w = nc.dram_tensor("idx1w", [P, sk1 * 8], I16, kind="ExternalInput")
    idx3w = nc.dram_tensor("idx3w", [P, sk3 * 8], I16, kind="ExternalInput")
    W1 = nc.dram_tensor("W1", [n_feat, D], F32, kind="ExternalInput")
    W2 = nc.dram_tensor("W2", [D, D], F32, kind="ExternalInput")
    W3 = nc.dram_tensor("W3", [D, D], F32, kind="ExternalInput")
    M1 = nc.dram_tensor("M1", [D, D], F32, kind="ExternalInput")
    M2 = nc.dram_tensor("M2", [D, D], F32, kind="ExternalInput")
    Wd0 = nc.dram_tensor("Wd0", [D, D], F32, kind="ExternalInput")
    Wc = nc.dram_tensor("Wc", [D, ncls], F32, kind="ExternalInput")
    out = nc.dram_tensor("out", [nloc, ncls + 2], F32, kind="ExternalOutput")

    t1_s = nc.dram_tensor("t1_s", [nloc, DD], TDT, kind="Internal")
    T1 = nc.dram_tensor("T1", [ng, DD], TDT, kind="Internal", addr_space=shared)
    zd_s = nc.dram_tensor("zd_s", [nloc, DD], TDT, kind="Internal")
    ZD = nc.dram_tensor("ZD", [ng, DD], TDT, kind="Internal", addr_space=shared)
    e1_s = nc.dram_tensor("e1_s", [nloc, D], F32, kind="Internal")
    e1h_s = nc.dram_tensor("e1h_s", [nloc, DD], TDT, kind="Internal")
    e1h_h = nc.dram_tensor("e1h_h", [nloc, DD], TDT, kind="Internal")
    e1d_s = nc.dram_tensor("e1d_s", [nloc, D], TDT, kind="Internal")
    E1H = nc.dram_tensor("E1H", [ng, DD], TDT, kind="Internal", addr_space=shared)
    E1D = nc.dram_tensor("E1D", [ng, D], TDT, kind="Internal", addr_space=shared)
    E2h = nc.dram_tensor("E2h", [nloc, DD], TDT, kind="Internal")
    TV = nc.dram_tensor("TV", [nloc, D], F32, kind="Internal")

    qc = [0]

    def next_q():
        q = qc[0]
        qc[0] = (qc[0] + 1) % NQ
        return q

    with tile.TileContext(nc) as tc:
        with (
            tc.tile_pool(name="const", bufs=1) as constp,
            tc.tile_pool(name="idxs", bufs=3) as idxp,
            tc.tile_pool(name="gath", bufs=3) as gathp,
            tc.tile_pool(name="lgath", bufs=2) as lgathp,
            tc.tile_pool(name="work", bufs=3) as workp,
            tc.tile_pool(name="outp", bufs=3) as outp,
            tc.tile_pool(name="psum", bufs=2, space="PSUM") as psp,
        ):
            ident = constp.tile([P, P], F32)
            make_identity(nc, ident[:])

            def res(t_dram, w, dt=F32, name=None):
                tl = constp.tile([P, w], dt, name=name)
                nc.sync.dma_start(tl[:], t_dram.ap())
                return tl

            disq = res(dis_p, nt, name="disq")
            dis2q = res(dis2_p, nt, name="dis2q")
            dishpq = res(dishp_p, nt, name="dishpq")
            dishhq = res(dishh_p, nt, name="dishhq")
            hordq = res(hordw, nloc // 16, I16, name="hordq")
            idxRq = res(idxRw, nloc // 16, I16, name="idxRq")

            w1t = [constp.tile([P, D], F32, name=f"w1t_{i}") for i in range(4)]
            for i in range(4):
                nc.sync.dma_start(w1t[i][:], W1.ap()[i * P : (i + 1) * P])
            w2t = res(W2, D, name="w2t")
            w3t = res(W3, D, name="w3t")
            m1t = res(M1, D, name="m1t")
            m2t = res(M2, D, name="m2t")
            wdt = res(Wd0, D, name="wdt")
            wct = res(Wc, ncls, name="wct")

            def rows(t):
                return slice(t * P, (t + 1) * P)

            def col(plane, t):
                return plane[:, t : t + 1]

            NSPL = 8
            bound = [nt * (i + 1) // NSPL - 1 for i in range(NSPL)]

            def ag_piece(src, dst, piece):
                r0 = (nt * piece // NSPL) * P
                r1 = (nt * (piece + 1) // NSPL) * P
                nc.gpsimd.collective_compute(
                    "AllGather",
                    ALU.bypass,
                    replica_groups=groups,
                    ins=[src[r0:r1].opt()],
                    outs=[dst[n_cores * r0 : n_cores * r1].opt()],
                )

            # ---- S0: T1 shard = [(x@W1)*dis | (x[perm]@W1)*dis]
            sp = 0
            for t in range(nt):
                xt = workp.tile([P, nfc2 * P], F32, tag="xt")
                nc.scalar.dma_start(xt[:], xTb.ap()[rows(t)])
                o = outp.tile([P, DD], TDT, tag="s0")
                for h in range(2):
                    ps = psp.tile([P, D], F32, tag="mm")
                    for i in range(4):
                        nc.tensor.matmul(
                            out=ps[:],
                            lhsT=xt[:, (h * 4 + i) * P : (h * 4 + i + 1) * P],
                            rhs=w1t[i][:],
                            start=(i == 0),
                            stop=(i == 3),
                        )
                    nc.vector.tensor_scalar_mul(
                        o[:, h * D : (h + 1) * D], ps[:], col(disq, t)
                    )
                nc.sync.dma_start(t1_s.ap()[rows(t)], o[:])
                if t == bound[sp]:
                    ag_piece(t1_s, T1, sp)
                    sp += 1

            # ---- chunked-ELL gather driver --------------------------------
            # one dma_gather per (tile, chunk): K[t,ch]*128 rows land
            # slot-major in g[:, off: off+K*w]; reduce over all slots + local
            # add as before. idx planes streamed per IDX_GRP tiles.
            def ell_run(table, width, Ks, idxw_dram, koff8, local_s, tail):
                sk = int(Ks.sum())
                for g0 in range(0, nt, IDX_GRP):
                    g1 = min(g0 + IDX_GRP, nt)
                    c0 = int(koff8[g0])
                    c1 = int(koff8[g1])
                    ip = idxp.tile([P, c1 - c0], I16, tag="ip")
                    nc.sync.dma_start(ip[:], idxw_dram.ap()[:, c0:c1])
                    for t in range(g0, g1):
                        Ktot = int(Ks[t].sum())
                        g = gathp.tile([P, Ktot * width], TDT, tag="ge")
                        off = 0
                        for ch in range(NCHUNK):
                            K = int(Ks[t, ch])
                            if K == 0:
                                continue
                            nidx = K * P
                            i0 = int(koff8[t] - c0 + Ks[t, :ch].sum() * (P // 16))
                            nc.gpsimd.dma_gather(
                                out_ap=g[:, off * width : (off + K) * width]
                                .rearrange("p (b w) -> p b w", w=width),
                                in_ap=table.ap()[
                                    cbase[ch] : cbase[ch] + csz[ch]
                                ],
                                idxs_ap=ip[:, i0 : i0 + nidx // 16],
                                num_idxs=nidx,
                                num_idxs_reg=nidx,
                                elem_size=width,
                                single_packet=False,
                                queue_num=next_q(),
                            )
                            off += K
                        s = workp.tile([P, width], F32, tag="se")
                        nc.vector.tensor_reduce(
                            out=s[:],
                            in_=g[:].rearrange("p (k d) -> p d k", k=Ktot),
                            axis=mybir.AxisListType.X,
                            op=ALU.add,
                        )
                        if local_s is not None:
                            li = workp.tile([P, width], TDT, tag="sl")
                            nc.scalar.dma_start(li[:], local_s.ap()[rows(t)])
                            nc.vector.tensor_tensor(
                                out=s[:], in0=s[:], in1=li[:], op=ALU.add
                            )
                        tail(t, s)

            # cumulative wrapped-col offsets per tile (8 = 128/16 wrap cols
            # per slot)
            def koff8_of(Ks):
                per_tile = Ks.sum(axis=1) * (P // 16)
                out_ = np.zeros(nt + 1, np.int64)
                out_[1:] = np.cumsum(per_tile)
                return out_

            koff8_1 = koff8_of(np.asarray(K1))
            koff8_3 = koff8_of(np.asarray(K3))

            # ---- G1: zd = relu(dis2 * (sum + t1_s)) -> zd_s
            spl = [0]

            def g1_tail(t, s):
                o = outp.tile([P, DD], TDT, tag="ze")
                nc.vector.tensor_scalar(
                    o[:], s[:], col(dis2q, t), 0.0, ALU.mult, ALU.max
                )
                nc.sync.dma_start(zd_s.ap()[rows(t)], o[:])
                if t == bound[spl[0]]:
                    ag_piece(zd_s, ZD, spl[0])
                    spl[0] += 1

            ell_run(T1, DD, np.asarray(K1), idx1w, koff8_1, t1_s, g1_tail)

            # ---- G2: S @ W2, three shipped variants
            def g2_tail(t, s):
                e1h = outp.tile([P, DD], TDT, tag="e1h")
                e1d = outp.tile([P, D], TDT, tag="e1d")
                e1p = outp.tile([P, D], F32, tag="e1p")
                for h in range(2):
                    tp = psp.tile([P, P], F32, tag="t", bufs=3)
                    nc.tensor.transpose(
                        out=tp[:], in_=s[:, h * D : (h + 1) * D], identity=ident[:]
                    )
                    tps = workp.tile([P, P], F32, tag="tps")
                    nc.vector.tensor_copy(tps[:], tp[:])
                    mm = psp.tile([P, D], F32, tag="m", bufs=3)
                    nc.tensor.matmul(
                        out=mm[:], lhsT=tps[:], rhs=w2t[:], start=True, stop=True
                    )
                    eh = workp.tile([P, D], F32, tag="eh")
                    nc.vector.tensor_scalar(
                        eh[:], mm[:], col(disq, t), 0.0, ALU.mult, ALU.max
                    )
                    nc.vector.tensor_scalar_mul(
                        e1h[:, h * D : (h + 1) * D], eh[:], col(dishpq, t)
                    )
                    if h == 0:
                        nc.vector.tensor_copy(e1p[:], eh[:])
                        nc.vector.tensor_scalar_mul(e1d[:], eh[:], col(disq, t))
                nc.sync.dma_start(e1_s.ap()[rows(t)], e1p[:])
                nc.sync.dma_start(e1h_s.ap()[rows(t)], e1h[:])
                nc.sync.dma_start(e1d_s.ap()[rows(t)], e1d[:])
                if t == bound[spl[0]]:
                    ag_piece(e1h_s, E1H, spl[0])
                    ag_piece(e1d_s, E1D, spl[0])
                    spl[0] += 1

            spl[0] = 0
            ell_run(ZD, DD, np.asarray(K1), idx1w, koff8_1, zd_s, g2_tail)

            # ---- H0: e1h_h = e1h_s[hord] (local gather, hop self-loop term)
            for t0 in range(0, nt, 24):
                t1 = min(t0 + 24, nt)
                nidx = (t1 - t0) * P
                gh = lgathp.tile([P, (t1 - t0) * DD], TDT, tag="gh")
                nc.gpsimd.dma_gather(
                    out_ap=gh[:].rearrange("p (b w) -> p b w", w=DD),
                    in_ap=e1h_s.ap(),
                    idxs_ap=hordq[:, t0 * 8 : t0 * 8 + nidx // 16],
                    num_idxs=nidx,
                    num_idxs_reg=nidx,
                    elem_size=DD,
                    single_packet=False,
                    queue_num=next_q(),
                )
                nc.sync.dma_start(e1h_h.ap()[t0 * P : t1 * P], gh[:])

            # ---- G3: embed2{,b} = dishh * (sum_hop + e1h_h) @ W3 -> E2h
            def g3_tail(t, s):
                e2 = outp.tile([P, DD], TDT, tag="e2")
                for h in range(2):
                    tp = psp.tile([P, P], F32, tag="t", bufs=3)
                    nc.tensor.transpose(
                        out=tp[:], in_=s[:, h * D : (h + 1) * D], identity=ident[:]
                    )
                    tps = workp.tile([P, P], F32, tag="tps")
                    nc.vector.tensor_copy(tps[:], tp[:])
                    mm = psp.tile([P, D], F32, tag="m", bufs=3)
                    nc.tensor.matmul(
                        out=mm[:], lhsT=tps[:], rhs=w3t[:], start=True, stop=True
                    )
                    nc.vector.tensor_scalar_mul(
                        e2[:, h * D : (h + 1) * D], mm[:], col(dishhq, t)
                    )
                nc.sync.dma_start(E2h.ap()[rows(t)], e2[:])

            ell_run(E1H, DD, np.asarray(K3), idx3w, koff8_3, e1h_h, g3_tail)

            # ---- S12: MLP + tvec
            for t in range(nt):
                et = workp.tile([P, D], F32, tag="ml_in")
                nc.sync.dma_start(et[:], e1_s.ap()[rows(t)])
                tp = psp.tile([P, P], F32, tag="t", bufs=3)
                nc.tensor.transpose(out=tp[:], in_=et[:], identity=ident[:])
                tps = workp.tile([P, P], F32, tag="tps")
                nc.vector.tensor_copy(tps[:], tp[:])
                mm = psp.tile([P, D], F32, tag="m", bufs=3)
                nc.tensor.matmul(out=mm[:], lhsT=tps[:], rhs=m1t[:], start=True, stop=True)
                u = workp.tile([P, D], F32, tag="ml_u")
                nc.scalar.activation(u[:], mm[:], AF.Relu)
                tp2 = psp.tile([P, P], F32, tag="t", bufs=3)
                nc.tensor.transpose(out=tp2[:], in_=u[:], identity=ident[:])
                tps2 = workp.tile([P, P], F32, tag="tps")
                nc.vector.tensor_copy(tps2[:], tp2[:])
                mm2 = psp.tile([P, D], F32, tag="m", bufs=3)
                nc.tensor.matmul(
                    out=mm2[:], lhsT=tps2[:], rhs=m2t[:], start=True, stop=True
                )
                e3 = workp.tile([P, D], F32, tag="ml_e3")
                nc.vector.tensor_copy(e3[:], mm2[:])
                tp3 = psp.tile([P, P], F32, tag="t", bufs=3)
                nc.tensor.transpose(out=tp3[:], in_=e3[:], identity=ident[:])
                tps3 = workp.tile([P, P], F32, tag="tps")
                nc.vector.tensor_copy(tps3[:], tp3[:])
                mm3 = psp.tile([P, D], F32, tag="m", bufs=3)
                nc.tensor.matmul(
                    out=mm3[:], lhsT=tps3[:], rhs=wdt[:], start=True, stop=True
                )
                tv = outp.tile([P, D], F32, tag="ml_tv")
                nc.vector.tensor_copy(tv[:], mm3[:])
                nc.sync.dma_start(TV.ap()[rows(t)], tv[:])

            # ---- S11: realign E2h to pi order + scores
            for t0 in range(0, nt, 24):
                t1 = min(t0 + 24, nt)
                nidx = (t1 - t0) * P
                gr = lgathp.tile([P, (t1 - t0) * DD], TDT, tag="gr")
                nc.gpsimd.dma_gather(
                    out_ap=gr[:].rearrange("p (b w) -> p b w", w=DD),
                    in_ap=E2h.ap(),
                    idxs_ap=idxRq[:, t0 * 8 : t0 * 8 + nidx // 16],
                    num_idxs=nidx,
                    num_idxs_reg=nidx,
                    elem_size=DD,
                    single_packet=False,
                    queue_num=next_q(),
                )
                for t in range(t0, t1):
                    e2 = gr[:, (t - t0) * DD : (t - t0 + 1) * DD]
                    tv = workp.tile([P, D], F32, tag="sc_tv")
                    nc.sync.dma_start(tv[:], TV.ap()[rows(t)])
                    pr = workp.tile([P, DD], F32, tag="sc_pr")
                    nc.vector.tensor_mul(pr[:, 0:D], tv[:], e2[:, 0:D])
                    nc.vector.tensor_mul(pr[:, D:DD], tv[:], e2[:, D:DD])
                    rs = workp.tile([P, 2], F32, tag="sc_rs")
                    nc.vector.tensor_reduce(
                        out=rs[:],
                        in_=pr[:].rearrange("p (h d) -> p h d", h=2),
                        axis=mybir.AxisListType.X,
                        op=ALU.add,
                    )
                    sg = outp.tile([P, 2], F32, tag="sc_sg")
                    nc.scalar.activation(sg[:], rs[:], AF.Sigmoid)
                    nc.sync.dma_start(out.ap()[rows(t), ncls : ncls + 2], sg[:])

            # ---- G4: cls = (dis * (sum + e1d_s)) @ Wc -> out[:, :ncls]
            def g4_tail(t, s):
                sc_ = workp.tile([P, D], F32, tag="c_s")
                nc.vector.tensor_scalar_mul(sc_[:], s[:], col(disq, t))
                tp = psp.tile([P, P], F32, tag="t", bufs=3)
                nc.tensor.transpose(out=tp[:], in_=sc_[:], identity=ident[:])
                tps = workp.tile([P, P], F32, tag="tps")
                nc.vector.tensor_copy(tps[:], tp[:])
                mm = psp.tile([P, ncls], F32, tag="m", bufs=3)
                nc.tensor.matmul(out=mm[:], lhsT=tps[:], rhs=wct[:], start=True, stop=True)
                o = outp.tile([P, ncls], F32, tag="c_o")
                nc.vector.tensor_copy(o[:], mm[:])
                nc.sync.dma_start(out.ap()[rows(t), 0:ncls], o[:])

            ell_run(E1D, D, np.asarray(K1), idx1w, koff8_1, e1d_s, g4_tail)

    nc.compile()
    return nc


def assemble(results, meta):
    n_cores = meta["n_cores"]
    N = len(meta["core_of"])
    ncls = meta["ncls"]
    out = np.empty((N, ncls + 2), np.float32)
    for c in range(n_cores):
        oc = results[c]["out"]
        m = meta["core_of"] == c
        out[m] = oc[meta["loc_of"][m]]
    return out


# ------------------------------------------------------------------ entry


_CACHE = {}
TRACE = False
LAST_RES = None


def kernel(**inputs):
    """Full-input entry point: shards across 8 NeuronCores internally."""
    n_cores = 8
    in_maps, meta = prep(inputs, n_cores)
    key = (meta["nloc"], meta["K1"].tobytes(), meta["K3"].tobytes())
    nc = _CACHE.get(key)
    if nc is None:
        nc = build(meta)
        _CACHE[key] = nc
    res = bass_utils.run_bass_kernel_spmd(
        nc, in_maps, core_ids=list(range(n_cores)), trace=TRACE
    )
    global LAST_RES
    LAST_RES = res
    return assemble(res.results, meta)


# revision 13
# speedup vs baseline: 1.4692x; 1.0153x over previous
"""8-core Trainium2 Bass kernel for nn_MixModel (GCN mix model) — v4.

v4 replaces v3's per-slot indirect_dma_start ELL gathers (994ns fixed cost per
128-row call = 8.5ns/row, single SWDGE queue ~65GB/s) with bulk dma_gather on
4 SWDGE queues (~175GB/s sustained on random 512B rows). dma_gather requires
int16 indices -> tables are split into 4 chunks (<=32768 rows, aligned to
AllGather piece pairs); per-(tile,chunk) ELL rectangles are padded to the
chunk max. Padding is cut from 2.2x to ~1.5x by grouping dst rows into tiles
by per-chunk in-degree profile (balanced KD split within each (core, quarter)
so table chunk membership stays fixed).

Other v4 changes vs v3:
 - bad path's first layer is computed from host-permuted x (xTb carries
   good|bad halves): kills AG0 (XW AllGather) + the gperm/ratio gather.
 - G3's hop self-loop slot becomes a local dma_gather (hord reorder of
   e1h_s) + local-tile add, so ELL rectangles stay chunk-pure.
 - S11's realign and the hop reorder use local (unchunked) dma_gather.
 - int16 wrapped index planes are streamed per tile group (not resident).

Stages (per core):
  S0   T1 shard = [(x@W1)*dis | (x[perm]@W1)*dis] ; AG1 -> T1 [NG,256]
  G1   ELL gather T1 -> zd = relu(dis^2 * (sum + t1_s))      -> AG2 ZD
  G2   ELL gather ZD -> S ; e1{,b} = relu(dis * (S_h @ W2)) ;
       ship [e1*dish|e1b*dish] -> AG3a E1H ; [e1*dis] -> AG3b E1D ; e1 local
  H0   e1h_h = e1h_s[hord] (local gather)
  G3   ELL gather E1H (hop order) + e1h_h -> embed2{,b} -> E2h (bf16)
  S12  MLP: embed3 = relu(e1@M1)@M2 ; tvec = embed3@Wd0
  S11  realign E2h to pi order (local gather); scores = sigmoid(...)
  G4   ELL gather E1D + e1d_s -> cls = (dis*sum)@Wc -> OUT[:, :10]
"""

import numpy as np

import concourse.bacc as bacc
import concourse.bass as bass
import concourse.mybir as mybir
import concourse.tile as tile
from concourse import bass_utils
from concourse.masks import make_identity

P = 128
F32 = mybir.dt.float32
I32 = mybir.dt.int32
I16 = mybir.dt.int16
AF = mybir.ActivationFunctionType
ALU = mybir.AluOpType
TDT = mybir.dt.bfloat16  # transport/table dtype

NCHUNK = 4
NQ = 4          # SWDGE queues
IDX_GRP = 16    # tiles per streamed idx-plane group

# ----------------------------------------------------------------- host prep


def _wrap16(flat):
    """[n] int64 -> wrapped [128, n//16] int16 (replicated across 8 groups)."""
    n = len(flat)
    w = flat.reshape(n // 16, 16).T.astype(np.int16)
    return np.tile(w, (8, 1))


def _kd_group(pr, n_tiles):
    """Split len(pr) rows into n_tiles groups of P by cycling-dim median.

    The split-dim sequence is FIXED (depth % NCHUNK) so all cores produce
    aligned profile regions per tile index -- the shared-K (max over cores)
    padding stays close to the per-core padding.
    """
    out = []

    def split(ii, k, d):
        if k == 1:
            out.append(ii)
            return
        k1 = k // 2
        o = np.argsort(pr[ii, d % NCHUNK], kind="stable")
        split(ii[o[: k1 * P]], k1, d + 1)
        split(ii[o[k1 * P :]], k - k1, d + 1)

    split(np.arange(len(pr)), n_tiles, 0)
    return out


def _ell_build_chunked(src_g, dst_core, dst_loc, n_cores, nloc, cbounds, padg):
    """Chunked shared-K ELL.

    Returns (K [nt, NCHUNK] per-chunk slot counts (same for all cores),
    per-core wrapped int16 idx planes [128, sum(K)*8] with chunk-local
    indices, slot-major per rectangle). padg[c][ch] = global row of a zero
    row inside chunk ch for core c.
    """
    nt = nloc // P
    cid = np.searchsorted(cbounds, src_g, side="right") - 1
    counts = np.zeros((n_cores, nloc, NCHUNK), np.int64)
    np.add.at(counts, (dst_core, dst_loc, cid), 1)
    K = counts.reshape(n_cores, nt, P, NCHUNK).max(axis=(0, 2))  # [nt, NCHUNK]

    order = np.lexsort((src_g, dst_loc, dst_core))
    sc, sl, sg = dst_core[order], dst_loc[order], src_g[order]
    scid = cid[order]
    key = ((sc * nloc + sl) * NCHUNK + scid).astype(np.int64)
    is_start = np.r_[True, key[1:] != key[:-1]] if len(key) else np.array([], bool)
    run_starts = np.flatnonzero(is_start)
    run_len = np.diff(np.r_[run_starts, len(key)])
    pos_in_run = np.arange(len(key)) - np.repeat(run_starts, run_len)

    koff = np.zeros((nt, NCHUNK), np.int64)
    flat = np.cumsum(K.reshape(-1))
    koff.reshape(-1)[1:] = flat[:-1]
    sk = int(flat[-1])

    planes = []
    for c in range(n_cores):
        # flat slot-major list per rectangle: entry (t, ch, k, p)
        fl = np.zeros(sk * P, np.int64)
        # fill pads first
        for t in range(nt):
            for ch in range(NCHUNK):
                base = koff[t, ch] * P
                fl[base : base + K[t, ch] * P] = padg[c][ch] - cbounds[ch]
        m = sc == c
        loc, pos, gidx, ch = sl[m], pos_in_run[m], sg[m], scid[m]
        t = loc // P
        p = loc % P
        fl[(koff[t, ch] + pos) * P + p] = gidx - cbounds[ch]
        planes.append(_wrap16(fl))
    return K, planes


def _plane(vals_loc, nt):
    return np.ascontiguousarray(vals_loc.reshape(nt, P).T)


def prep(inputs, n_cores=8):
    x = np.asarray(inputs["x"], np.float32)
    ei = np.asarray(inputs["edge_index"], np.int64)
    eih = np.asarray(inputs["edge_index_hop"], np.int64)
    perm = np.asarray(inputs["perm"], np.int64)
    W1 = np.asarray(inputs["W1"], np.float32)
    W2 = np.asarray(inputs["W2"], np.float32)
    W3 = np.asarray(inputs["W3"], np.float32)
    M1 = np.asarray(inputs["M1"], np.float32)
    M2 = np.asarray(inputs["M2"], np.float32)
    Wc = np.asarray(inputs["Wc"], np.float32)
    Wd0 = np.asarray(inputs["Wd"], np.float32)[0]
    for bname in ("b1", "b2", "b3", "mb1", "mb2", "bc"):
        assert np.abs(np.asarray(inputs[bname])).max() == 0.0, (
            f"nonzero bias {bname} not supported by this kernel build"
        )

    N, n_feat = x.shape
    D = W1.shape[1]
    ncls = Wc.shape[1]
    max_real = -(-N // n_cores)
    nloc = -(-(max_real + 1) // P) * P
    nt = nloc // P
    ng = n_cores * nloc

    deg = np.bincount(ei[1], minlength=N).astype(np.float32) + 1.0
    degh = np.bincount(eih[1], minlength=N).astype(np.float32) + 1.0
    dis = 1.0 / np.sqrt(deg)
    dish = 1.0 / np.sqrt(degh)

    order = np.argsort(-deg, kind="stable")
    core_of = np.empty(N, np.int64)
    loc_of = np.empty(N, np.int64)
    core_of[order] = np.arange(N) % n_cores
    loc_of[order] = np.arange(N) // n_cores

    # piece-major global layout; chunks = piece pairs (<=32768 rows, int16).
    NSPL = 8
    pr0 = np.array([(nt * i // NSPL) * P for i in range(NSPL + 1)], np.int64)
    qb = [0, int(pr0[2]), int(pr0[4]), int(pr0[6]), nloc]

    # remap dense ranks so each quarter keeps >=1 unassigned slot (zero row)
    reserved = [qb[1] - 1, qb[2] - 1, qb[3] - 1]
    avail = np.setdiff1d(np.arange(nloc), np.array(reserved, np.int64))
    loc_of = avail[loc_of]

    def glmap(c, r):
        p = np.searchsorted(pr0, r, side="right") - 1
        return 8 * pr0[p] + c * (pr0[p + 1] - pr0[p]) + (r - pr0[p])

    cbounds = (8 * pr0)[::2][:NCHUNK]  # chunk start rows (global)

    # chunk profile of each dst node (chunk membership is loc-permutation
    # invariant within quarters, so profiles stay exact through regrouping)
    gl0 = glmap(core_of, loc_of)
    cid1 = np.searchsorted(cbounds, gl0[ei[0]], side="right") - 1
    prof1 = np.zeros((N, NCHUNK), np.int32)
    np.add.at(prof1, (ei[1], cid1), 1)

    # regroup: within each (core, quarter), KD-group the real rows by chunk
    # profile; at least one slot per quarter stays unassigned (zero pad row).
    loc_new = np.full(N, -1, np.int64)
    padloc = np.zeros((n_cores, NCHUNK), np.int64)
    for c in range(n_cores):
        nodes_c = np.flatnonzero(core_of == c)
        locs_c = loc_of[nodes_c]
        for qi in range(NCHUNK):
            lo, hi = qb[qi], qb[qi + 1]
            seg = nodes_c[(locs_c >= lo) & (locs_c < hi)]
            navail = hi - lo
            assert len(seg) <= navail - 1, (len(seg), navail)
            ntile = navail // P
            pr = prof1[seg].astype(np.int32)
            prf = np.vstack([pr, np.zeros((navail - len(seg), NCHUNK), np.int32)])
            groups = _kd_group(prf, ntile)
            used = np.zeros(navail, bool)
            pos = lo
            for g in groups:
                real = g[g < len(seg)]
                loc_new[seg[real]] = pos + np.arange(len(real))
                used[pos - lo : pos - lo + len(real)] = True
                pos += len(g)
            free_slots = np.flatnonzero(~used)
            assert len(free_slots) >= 1
            padloc[c, qi] = lo + free_slots[0]
    loc_of = loc_new
    assert (loc_of >= 0).all()

    gl = glmap(core_of, loc_of)
    padg = [[glmap(np.int64(c), np.int64(padloc[c, q])) for q in range(NCHUNK)]
            for c in range(n_cores)]

    nat = np.full((n_cores, nloc), -1, np.int64)
    nat[core_of, loc_of] = np.arange(N)

    # hop order: free per-core regroup by hop chunk profile
    cidh = np.searchsorted(cbounds, gl[eih[0]], side="right") - 1
    profh = np.zeros((N, NCHUNK), np.int32)
    np.add.at(profh, (eih[1], cidh), 1)
    hord = np.empty((n_cores, nloc), np.int64)  # hop row r <- pi-loc hord[c,r]
    for c in range(n_cores):
        natc = nat[c]
        pr = np.zeros((nloc, NCHUNK), np.int32)
        rm = natc >= 0
        pr[rm] = profh[natc[rm]]
        groups = _kd_group(pr, nt)
        perm_rows = np.concatenate(groups)
        hord[c] = perm_rows
    hpos = np.argsort(hord, axis=1)

    K1, idx1p = _ell_build_chunked(
        gl[ei[0]], core_of[ei[1]], loc_of[ei[1]], n_cores, nloc, cbounds, padg
    )
    K3, idx3p = _ell_build_chunked(
        gl[eih[0]],
        core_of[eih[1]],
        hpos[core_of[eih[1]], loc_of[eih[1]]],
        n_cores,
        nloc,
        cbounds,
        padg,
    )

    in_maps = []
    for c in range(n_cores):
        natc = nat[c]
        real = natc >= 0
        xs = np.zeros((nloc, n_feat), np.float32)
        xs[real] = x[natc[real]]
        xb = np.zeros((nloc, n_feat), np.float32)
        xb[real] = x[perm[natc[real]]]
        dis_c = np.ones(nloc, np.float32)
        dis_c[real] = dis[natc[real]]
        dish_pi = np.ones(nloc, np.float32)
        dish_pi[real] = dish[natc[real]]
        dishh = np.ones(nloc, np.float32)
        hnat = natc[hord[c]]
        hreal = hnat >= 0
        dishh[hreal] = dish[hnat[hreal]]
        xcat = np.concatenate([xs, xb], axis=1)  # [nloc, 2*n_feat]
        nfc2 = 2 * n_feat // P
        in_maps.append(
            {
                "xTb": np.ascontiguousarray(
                    xcat.reshape(nt, P, nfc2, P).transpose(0, 3, 2, 1)
                    .reshape(nt * P, nfc2 * P)
                ),
                "dis_p": _plane(dis_c, nt),
                "dis2_p": _plane(dis_c * dis_c, nt),
                "dishp_p": _plane(dish_pi, nt),
                "dishh_p": _plane(dishh, nt),
                "hordw": _wrap16(hord[c]),
                "idxRw": _wrap16(hpos[c]),
                "idx1w": idx1p[c],
                "idx3w": idx3p[c],
                "W1": W1,
                "W2": W2,
                "W3": W3,
                "M1": M1,
                "M2": M2,
                "Wd0": Wd0,
                "Wc": np.ascontiguousarray(Wc),
            }
        )

    meta = dict(
        n_cores=n_cores,
        nloc=nloc,
        nt=nt,
        ng=ng,
        n_feat=n_feat,
        D=D,
        ncls=ncls,
        K1=K1,
        K3=K3,
        pr0=pr0,
        core_of=core_of,
        loc_of=loc_of,
    )
    return in_maps, meta


# ------------------------------------------------------------- device build


def build(meta):
    n_cores = meta["n_cores"]
    nloc, nt, ng = meta["nloc"], meta["nt"], meta["ng"]
    n_feat, D, ncls = meta["n_feat"], meta["D"], meta["ncls"]
    K1, K3 = meta["K1"], meta["K3"]
    pr0 = meta["pr0"]
    DD = 2 * D
    nfc2 = 2 * n_feat // P
    sk1 = int(K1.sum())
    sk3 = int(K3.sum())
    groups = [list(range(n_cores))]
    csz = [int(8 * (pr0[2 * i + 2] - pr0[2 * i])) for i in range(NCHUNK)]
    cbase = [int((8 * pr0)[::2][i]) for i in range(NCHUNK)]

    nc = bacc.Bacc(
        "TRN2", debug=False, num_devices=n_cores, num_swdge_queues=NQ
    )
    shared = "Shared" if n_cores > 4 else "Local"

    xTb = nc.dram_tensor("xTb", [nt * P, nfc2 * P], F32, kind="ExternalInput")
    dis_p = nc.dram_tensor("dis_p", [P, nt], F32, kind="ExternalInput")
    dis2_p = nc.dram_tensor("dis2_p", [P, nt], F32, kind="ExternalInput")
    dishp_p = nc.dram_tensor("dishp_p", [P, nt], F32, kind="ExternalInput")
    dishh_p = nc.dram_tensor("dishh_p", [P, nt], F32, kind="ExternalInput")
    hordw = nc.dram_tensor("hordw", [P, nloc // 16], I16, kind="ExternalInput")
    idxRw = nc.dram_tensor("idxRw", [P, nloc // 16], I16, kind="ExternalInput")
    idx1w = nc.dram_tensor("idx1w", [P, sk1 * 8], I16, kind="ExternalInput")
    idx3w = nc.dram_tensor("idx3w", [P, sk3 * 8], I16, kind="ExternalInput")
    W1 = nc.dram_tensor("W1", [n_feat, D], F32, kind="ExternalInput")
    W2 = nc.dram_tensor("W2", [D, D], F32, kind="ExternalInput")
    W3 = nc.dram_tensor("W3", [D, D], F32, kind="ExternalInput")
    M1 = nc.dram_tensor("M1", [D, D], F32, kind="ExternalInput")
    M2 = nc.dram_tensor("M2", [D, D], F32, kind="ExternalInput")
    Wd0 = nc.dram_tensor("Wd0", [D, D], F32, kind="ExternalInput")
    Wc = nc.dram_tensor("Wc", [D, ncls], F32, kind="ExternalInput")
    out = nc.dram_tensor("out", [nloc, ncls + 2], F32, kind="ExternalOutput")

    t1_s = nc.dram_tensor("t1_s", [nloc, DD], TDT, kind="Internal")
    T1 = nc.dram_tensor("T1", [ng, DD], TDT, kind="Internal", addr_space=shared)
    zd_s = nc.dram_tensor("zd_s", [nloc, DD], TDT, kind="Internal")
    ZD = nc.dram_tensor("ZD", [ng, DD], TDT, kind="Internal", addr_space=shared)
    e1_s = nc.dram_tensor("e1_s", [nloc, D], F32, kind="Internal")
    e1h_s = nc.dram_tensor("e1h_s", [nloc, DD], TDT, kind="Internal")
    e1h_h = nc.dram_tensor("e1h_h", [nloc, DD], TDT, kind="Internal")
    e1d_s = nc.dram_tensor("e1d_s", [nloc, D], TDT, kind="Internal")
    E1H = nc.dram_tensor("E1H", [ng, DD], TDT, kind="Internal", addr_space=shared)
    E1D = nc.dram_tensor("E1D", [ng, D], TDT, kind="Internal", addr_space=shared)
    E2h = nc.dram_tensor("E2h", [nloc, DD], TDT, kind="Internal")
    TV = nc.dram_tensor("TV", [nloc, D], F32, kind="Internal")

    qc = [0]

    def next_q():
        q = qc[0]
        qc[0] = (qc[0] + 1) % NQ
        return q

    with tile.TileContext(nc) as tc:
        with (
            tc.tile_pool(name="const", bufs=1) as constp,
            tc.tile_pool(name="idxs", bufs=3) as idxp,
            tc.tile_pool(name="gath", bufs=3) as gathp,
            tc.tile_pool(name="lgath", bufs=2) as lgathp,
            tc.tile_pool(name="work", bufs=3) as workp,
            tc.tile_pool(name="outp", bufs=3) as outp,
            tc.tile_pool(name="psum", bufs=2, space="PSUM") as psp,
        ):
            ident = constp.tile([P, P], F32)
            make_identity(nc, ident[:])

            def res(t_dram, w, dt=F32, name=None):
                tl = constp.tile([P, w], dt, name=name)
                nc.sync.dma_start(tl[:], t_dram.ap())
                return tl

            disq = res(dis_p, nt, name="disq")
            dis2q = res(dis2_p, nt, name="dis2q")
            dishpq = res(dishp_p, nt, name="dishpq")
            dishhq = res(dishh_p, nt, name="dishhq")
            hordq = res(hordw, nloc // 16, I16, name="hordq")
            idxRq = res(idxRw, nloc // 16, I16, name="idxRq")

            w1t = [constp.tile([P, D], F32, name=f"w1t_{i}") for i in range(4)]
            for i in range(4):
                nc.sync.dma_start(w1t[i][:], W1.ap()[i * P : (i + 1) * P])
            w2t = res(W2, D, name="w2t")
            w3t = res(W3, D, name="w3t")
            m1t = res(M1, D, name="m1t")
            m2t = res(M2, D, name="m2t")
            wdt = res(Wd0, D, name="wdt")
            wct = res(Wc, ncls, name="wct")

            def rows(t):
                return slice(t * P, (t + 1) * P)

            def col(plane, t):
                return plane[:, t : t + 1]

            NSPL = 8
            bound = [nt * (i + 1) // NSPL - 1 for i in range(NSPL)]

            def ag_piece(src, dst, piece):
                r0 = (nt * piece // NSPL) * P
                r1 = (nt * (piece + 1) // NSPL) * P
                nc.gpsimd.collective_compute(
                    "AllGather",
                    ALU.bypass,
                    replica_groups=groups,
                    ins=[src[r0:r1].opt()],
                    outs=[dst[n_cores * r0 : n_cores * r1].opt()],
                )

            # ---- S0: T1 shard = [(x@W1)*dis | (x[perm]@W1)*dis]
            sp = 0
            for t in range(nt):
                xt = workp.tile([P, nfc2 * P], F32, tag="xt")
                nc.scalar.dma_start(xt[:], xTb.ap()[rows(t)])
                o = outp.tile([P, DD], TDT, tag="s0")
                for h in range(2):
                    ps = psp.tile([P, D], F32, tag="mm")
                    for i in range(4):
                        nc.tensor.matmul(
                            out=ps[:],
                            lhsT=xt[:, (h * 4 + i) * P : (h * 4 + i + 1) * P],
                            rhs=w1t[i][:],
                            start=(i == 0),
                            stop=(i == 3),
                        )
                    nc.scalar.activation(
                        o[:, h * D : (h + 1) * D], ps[:], AF.Identity,
                        scale=col(disq, t),
                    )
                nc.sync.dma_start(t1_s.ap()[rows(t)], o[:])
                if t == bound[sp]:
                    ag_piece(t1_s, T1, sp)
                    sp += 1

            # ---- chunked-ELL gather driver --------------------------------
            # one dma_gather per (tile, chunk): K[t,ch]*128 rows land
            # slot-major in g[:, off: off+K*w]; reduce over all slots + local
            # add as before. idx planes streamed per IDX_GRP tiles.
            def ell_run(table, width, Ks, idxw_dram, koff8, local_s, tail):
                sk = int(Ks.sum())
                for g0 in range(0, nt, IDX_GRP):
                    g1 = min(g0 + IDX_GRP, nt)
                    c0 = int(koff8[g0])
                    c1 = int(koff8[g1])
                    ip = idxp.tile([P, c1 - c0], I16, tag="ip")
                    nc.sync.dma_start(ip[:], idxw_dram.ap()[:, c0:c1])
                    for t in range(g0, g1):
                        Ktot = int(Ks[t].sum())
                        g = gathp.tile([P, Ktot * width], TDT, tag="ge")
                        off = 0
                        for ch in range(NCHUNK):
                            K = int(Ks[t, ch])
                            if K == 0:
                                continue
                            nidx = K * P
                            i0 = int(koff8[t] - c0 + Ks[t, :ch].sum() * (P // 16))
                            nc.gpsimd.dma_gather(
                                out_ap=g[:, off * width : (off + K) * width]
                                .rearrange("p (b w) -> p b w", w=width),
                                in_ap=table.ap()[
                                    cbase[ch] : cbase[ch] + csz[ch]
                                ],
                                idxs_ap=ip[:, i0 : i0 + nidx // 16],
                                num_idxs=nidx,
                                num_idxs_reg=nidx,
                                elem_size=width,
                                single_packet=False,
                                queue_num=next_q(),
                            )
                            off += K
                        s = workp.tile([P, width], F32, tag="se")
                        nc.vector.tensor_reduce(
                            out=s[:],
                            in_=g[:].rearrange("p (k d) -> p d k", k=Ktot),
                            axis=mybir.AxisListType.X,
                            op=ALU.add,
                        )
                        if local_s is not None:
                            li = workp.tile([P, width], TDT, tag="sl")
                            nc.scalar.dma_start(li[:], local_s.ap()[rows(t)])
                            nc.vector.tensor_tensor(
                                out=s[:], in0=s[:], in1=li[:], op=ALU.add
                            )
                        tail(t, s)

            # cumulative wrapped-col offsets per tile (8 = 128/16 wrap cols
            # per slot)
            def koff8_of(Ks):
                per_tile = Ks.sum(axis=1) * (P // 16)
                out_ = np.zeros(nt + 1, np.int64)
                out_[1:] = np.cumsum(per_tile)
                return out_

            koff8_1 = koff8_of(np.asarray(K1))
            koff8_3 = koff8_of(np.asarray(K3))

            # ---- G1: zd = relu(dis2 * (sum + t1_s)) -> zd_s
            spl = [0]

            def g1_tail(t, s):
                o = outp.tile([P, DD], TDT, tag="ze")
                nc.scalar.activation(o[:], s[:], AF.Relu, scale=col(dis2q, t))
                nc.sync.dma_start(zd_s.ap()[rows(t)], o[:])
                if t == bound[spl[0]]:
                    ag_piece(zd_s, ZD, spl[0])
                    spl[0] += 1

            ell_run(T1, DD, np.asarray(K1), idx1w, koff8_1, t1_s, g1_tail)

            # ---- G2: S @ W2, three shipped variants
            def g2_tail(t, s):
                e1h = outp.tile([P, DD], TDT, tag="e1h")
                e1d = outp.tile([P, D], TDT, tag="e1d")
                e1p = outp.tile([P, D], F32, tag="e1p")
                for h in range(2):
                    tp = psp.tile([P, P], F32, tag="t", bufs=3)
                    nc.tensor.transpose(
                        out=tp[:], in_=s[:, h * D : (h + 1) * D], identity=ident[:]
                    )
                    tps = workp.tile([P, P], F32, tag="tps")
                    nc.scalar.activation(tps[:], tp[:], AF.Identity)
                    mm = psp.tile([P, D], F32, tag="m", bufs=3)
                    nc.tensor.matmul(
                        out=mm[:], lhsT=tps[:], rhs=w2t[:], start=True, stop=True
                    )
                    eh = workp.tile([P, D], F32, tag="eh")
                    nc.scalar.activation(eh[:], mm[:], AF.Relu, scale=col(disq, t))
                    nc.scalar.activation(
                        e1h[:, h * D : (h + 1) * D], eh[:], AF.Identity,
                        scale=col(dishpq, t),
                    )
                    if h == 0:
                        nc.vector.tensor_copy(e1p[:], eh[:])
                        nc.scalar.activation(
                            e1d[:], eh[:], AF.Identity, scale=col(disq, t)
                        )
                nc.sync.dma_start(e1_s.ap()[rows(t)], e1p[:])
                nc.sync.dma_start(e1h_s.ap()[rows(t)], e1h[:])
                nc.sync.dma_start(e1d_s.ap()[rows(t)], e1d[:])
                if t == bound[spl[0]]:
                    ag_piece(e1h_s, E1H, spl[0])
                    ag_piece(e1d_s, E1D, spl[0])
                    spl[0] += 1

            spl[0] = 0
            ell_run(ZD, DD, np.asarray(K1), idx1w, koff8_1, zd_s, g2_tail)

            # ---- H0: e1h_h = e1h_s[hord] (local gather, hop self-loop term)
            for t0 in range(0, nt, 24):
                t1 = min(t0 + 24, nt)
                nidx = (t1 - t0) * P
                gh = lgathp.tile([P, (t1 - t0) * DD], TDT, tag="gh")
                nc.gpsimd.dma_gather(
                    out_ap=gh[:].rearrange("p (b w) -> p b w", w=DD),
                    in_ap=e1h_s.ap(),
                    idxs_ap=hordq[:, t0 * 8 : t0 * 8 + nidx // 16],
                    num_idxs=nidx,
                    num_idxs_reg=nidx,
                    elem_size=DD,
                    single_packet=False,
                    queue_num=next_q(),
                )
                nc.sync.dma_start(e1h_h.ap()[t0 * P : t1 * P], gh[:])

            # ---- G3: embed2{,b} = dishh * (sum_hop + e1h_h) @ W3 -> E2h
            def g3_tail(t, s):
                e2 = outp.tile([P, DD], TDT, tag="e2")
                for h in range(2):
                    tp = psp.tile([P, P], F32, tag="t", bufs=3)
                    nc.tensor.transpose(
                        out=tp[:], in_=s[:, h * D : (h + 1) * D], identity=ident[:]
                    )
                    tps = workp.tile([P, P], F32, tag="tps")
                    nc.scalar.activation(tps[:], tp[:], AF.Identity)
                    mm = psp.tile([P, D], F32, tag="m", bufs=3)
                    nc.tensor.matmul(
                        out=mm[:], lhsT=tps[:], rhs=w3t[:], start=True, stop=True
                    )
                    nc.scalar.activation(
                        e2[:, h * D : (h + 1) * D], mm[:], AF.Identity,
                        scale=col(dishhq, t),
                    )
                nc.sync.dma_start(E2h.ap()[rows(t)], e2[:])

            ell_run(E1H, DD, np.asarray(K3), idx3w, koff8_3, e1h_h, g3_tail)

            # ---- S12: MLP + tvec
            for t in range(nt):
                et = workp.tile([P, D], F32, tag="ml_in")
                nc.sync.dma_start(et[:], e1_s.ap()[rows(t)])
                tp = psp.tile([P, P], F32, tag="t", bufs=3)
                nc.tensor.transpose(out=tp[:], in_=et[:], identity=ident[:])
                tps = workp.tile([P, P], F32, tag="tps")
                nc.scalar.activation(tps[:], tp[:], AF.Identity)
                mm = psp.tile([P, D], F32, tag="m", bufs=3)
                nc.tensor.matmul(out=mm[:], lhsT=tps[:], rhs=m1t[:], start=True, stop=True)
                u = workp.tile([P, D], F32, tag="ml_u")
                nc.scalar.activation(u[:], mm[:], AF.Relu)
                tp2 = psp.tile([P, P], F32, tag="t", bufs=3)
                nc.tensor.transpose(out=tp2[:], in_=u[:], identity=ident[:])
                tps2 = workp.tile([P, P], F32, tag="tps")
                nc.scalar.activation(tps2[:], tp2[:], AF.Identity)
                mm2 = psp.tile([P, D], F32, tag="m", bufs=3)
                nc.tensor.matmul(
                    out=mm2[:], lhsT=tps2[:], rhs=m2t[:], start=True, stop=True
                )
                e3 = workp.tile([P, D], F32, tag="ml_e3")
                nc.scalar.activation(e3[:], mm2[:], AF.Identity)
                tp3 = psp.tile([P, P], F32, tag="t", bufs=3)
                nc.tensor.transpose(out=tp3[:], in_=e3[:], identity=ident[:])
                tps3 = workp.tile([P, P], F32, tag="tps")
                nc.scalar.activation(tps3[:], tp3[:], AF.Identity)
                mm3 = psp.tile([P, D], F32, tag="m", bufs=3)
                nc.tensor.matmul(
                    out=mm3[:], lhsT=tps3[:], rhs=wdt[:], start=True, stop=True
                )
                tv = outp.tile([P, D], F32, tag="ml_tv")
                nc.scalar.activation(tv[:], mm3[:], AF.Identity)
                nc.sync.dma_start(TV.ap()[rows(t)], tv[:])

            # ---- S11: realign E2h to pi order + scores
            for t0 in range(0, nt, 24):
                t1 = min(t0 + 24, nt)
                nidx = (t1 - t0) * P
                gr = lgathp.tile([P, (t1 - t0) * DD], TDT, tag="gr")
                nc.gpsimd.dma_gather(
                    out_ap=gr[:].rearrange("p (b w) -> p b w", w=DD),
                    in_ap=E2h.ap(),
                    idxs_ap=idxRq[:, t0 * 8 : t0 * 8 + nidx // 16],
                    num_idxs=nidx,
                    num_idxs_reg=nidx,
                    elem_size=DD,
                    single_packet=False,
                    queue_num=next_q(),
                )
                for t in range(t0, t1):
                    e2 = gr[:, (t - t0) * DD : (t - t0 + 1) * DD]
                    tv = workp.tile([P, D], F32, tag="sc_tv")
                    nc.sync.dma_start(tv[:], TV.ap()[rows(t)])
                    pr = workp.tile([P, DD], F32, tag="sc_pr")
                    nc.vector.tensor_mul(pr[:, 0:D], tv[:], e2[:, 0:D])
                    nc.vector.tensor_mul(pr[:, D:DD], tv[:], e2[:, D:DD])
                    rs = workp.tile([P, 2], F32, tag="sc_rs")
                    nc.vector.tensor_reduce(
                        out=rs[:],
                        in_=pr[:].rearrange("p (h d) -> p h d", h=2),
                        axis=mybir.AxisListType.X,
                        op=ALU.add,
                    )
                    sg = outp.tile([P, 2], F32, tag="sc_sg")
                    nc.scalar.activation(sg[:], rs[:], AF.Sigmoid)
                    nc.sync.dma_start(out.ap()[rows(t), ncls : ncls + 2], sg[:])

            # ---- G4: cls = (dis * (sum + e1d_s)) @ Wc -> out[:, :ncls]
            def g4_tail(t, s):
                sc_ = workp.tile([P, D], F32, tag="c_s")
                nc.scalar.activation(sc_[:], s[:], AF.Identity, scale=col(disq, t))
                tp = psp.tile([P, P], F32, tag="t", bufs=3)
                nc.tensor.transpose(out=tp[:], in_=sc_[:], identity=ident[:])
                tps = workp.tile([P, P], F32, tag="tps")
                nc.scalar.activation(tps[:], tp[:], AF.Identity)
                mm = psp.tile([P, ncls], F32, tag="m", bufs=3)
                nc.tensor.matmul(out=mm[:], lhsT=tps[:], rhs=wct[:], start=True, stop=True)
                o = outp.tile([P, ncls], F32, tag="c_o")
                nc.scalar.activation(o[:], mm[:], AF.Identity)
                nc.sync.dma_start(out.ap()[rows(t), 0:ncls], o[:])

            ell_run(E1D, D, np.asarray(K1), idx1w, koff8_1, e1d_s, g4_tail)

    nc.compile()
    return nc


def assemble(results, meta):
    n_cores = meta["n_cores"]
    N = len(meta["core_of"])
    ncls = meta["ncls"]
    out = np.empty((N, ncls + 2), np.float32)
    for c in range(n_cores):
        oc = results[c]["out"]
        m = meta["core_of"] == c
        out[m] = oc[meta["loc_of"][m]]
    return out


# ------------------------------------------------------------------ entry


_CACHE = {}
TRACE = False
LAST_RES = None


def kernel(**inputs):
    """Full-input entry point: shards across 8 NeuronCores internally."""
    n_cores = 8
    in_maps, meta = prep(inputs, n_cores)
    key = (meta["nloc"], meta["K1"].tobytes(), meta["K3"].tobytes())
    nc = _CACHE.get(key)
    if nc is None:
        nc = build(meta)
        _CACHE[key] = nc
    res = bass_utils.run_bass_kernel_spmd(
        nc, in_maps, core_ids=list(range(n_cores)), trace=TRACE
    )
    global LAST_RES
    LAST_RES = res
    return assemble(res.results, meta)
